# revision 19
# baseline (speedup 1.0000x reference)
"""Trainium2 Bass kernel for nn_Block_27187142983954 (dense transformer block,
per-position head-mixing attention). Data-parallel over batch: 8 cores, one
batch element each. Self-contained: hardcodes all shapes.

Per-core plan (S=4096 positions, E=1024, H=16 heads, D=64):
  - qkv projection on TensorE: stationary = x feature-major tiles (from a
    host-pretransposed bf16 copy of x), moving = host-pretransposed weight
    columns; biases and the first residual are folded into the PSUM
    accumulation as rank-1 (K=1 ones x bias) and identity-stationary matmuls.
  - attention (per-position bilinear over heads) on VectorE in position-major
    layout with broadcast access patterns: bf16 tensor_tensor muls in 2x mode
    and full halving-tree TT adds (also 2x) instead of 1x tensor_reduce.
  - softmax without max-subtraction (scores are O(1) by construction); the
    1/denominator is applied after the attn@v contraction (linearity), on
    GpSimd.
  - v is computed with host-permuted weight rows so its features land in
    (d,g) order, which keeps every broadcast AP's innermost dim contiguous.
  - activations are moved to feature-major for the proj/ff matmuls with
    DMA-engine xbar transposes (frees VectorE and TensorE).
  - LayerNorm stats on ScalarE via activation accum_out (Identity/Square)
    reading straight from PSUM; rsigma = exp(-0.5*ln(var+eps)) so softmax-exp
    and LN share one ACT table set; LN1's ln_g/ln_b are folded into the ff
    weights on the host; LN2's affine runs on GpSimd.
"""

import sys

sys.path.insert(0, "/opt/trn_rl_repo")

import numpy as np
import ml_dtypes

E, H, DQ, DV = 1024, 16, 64, 64
B, S = 8, 4096
EPS = 1e-5
NT = S // 128  # 32 position tiles per core
BF = ml_dtypes.bfloat16

_CACHE = {}


def _patch_tail_drain():
    """walrus in this container rejects >1 sem wait on a CTRL (Drain)
    instruction; spread the TileContext tail-drain waits over wait-nops."""
    import concourse.tile as tile
    import bass_rust
    from concourse.vector_clock import ScopedClock

    if getattr(tile.TileContext, "_drain_patched", False):
        return

    def _drain_and_barrier(self, tick_clock, wait_clock):
        nc = self.nc
        drain_inst = nc.sync.drain()
        wait_clock.add_sem_waits(
            drain_inst.ins, ScopedClock({None: tick_clock.global_clock})
        )
        si = drain_inst.ins.sync_info
        waits = list(si.on_wait) if si is not None else []
        if len(waits) > 1:
            drain_inst.ins.sync_info = bass_rust.SyncInfo(on_wait=[], on_update=[])
            for w in waits:
                nop = nc.sync.nop()
                nop.ins.sync_info = bass_rust.SyncInfo(on_wait=[w], on_update=[])
        nc.all_engine_barrier()
        assert self.sems is not None
        popped = nc._tile_sem_poison_stack.pop()
        assert popped is self._sem_poison
        nc.clear_and_free_semaphores(list(self.sems.allocated().values()))
        nc.all_engine_barrier()

    tile.TileContext._drain_and_barrier = _drain_and_barrier
    tile.TileContext._drain_patched = True


def _split_excess_waits(nc, max_on_op=1, max_on_nop=1):
    """walrus in this container rejects >1 sem wait per instruction struct.
    Hoist excess waits onto preceding same-engine NOPs."""
    import concourse.mybir as mybir
    import bass_rust

    cnt = 0
    for bb in nc.m.functions[0].blocks:
        il = bb.instructions
        out = []
        for inst in il:
            si = inst.sync_info
            waits = list(si.on_wait) if si is not None and si.on_wait else []
            if len(waits) > max_on_op:
                n_extra = len(waits) - max_on_op
                extra, keep = waits[:n_extra], waits[n_extra:]
                for i0 in range(0, len(extra), max_on_nop):
                    chunk = extra[i0 : i0 + max_on_nop]
                    nop = mybir.InstNoOp(name=f"waitnop-{cnt}", ins=[], outs=[])
                    cnt += 1
                    nop.engine = inst.engine
                    nop.sync_info = bass_rust.SyncInfo(on_wait=chunk, on_update=[])
                    out.append(nop)
                inst.sync_info = bass_rust.SyncInfo(
                    on_wait=keep,
                    on_update=list(si.on_update) if si.on_update else [],
                )
            out.append(inst)
        il[:] = out


def _build_program():
    import concourse.bass as bass
    import concourse.tile as tile
    import concourse.mybir as mybir
    from concourse.masks import make_identity

    _patch_tail_drain()

    f32 = mybir.dt.float32
    bf16 = mybir.dt.bfloat16
    ALU = mybir.AluOpType
    ACT = mybir.ActivationFunctionType

    nc = bass.Bass("TRN2", target_bir_lowering=False, debug=False, num_devices=1)

    x_pm = nc.dram_tensor("x_pm", [S, E], f32, kind="ExternalInput").ap()
    x_bf = nc.dram_tensor("x_bf", [S, E], bf16, kind="ExternalInput").ap()
    xT = nc.dram_tensor("xT", [E, S], bf16, kind="ExternalInput").ap()
    wqkvT_d = nc.dram_tensor("wqkvT", [E, 3 * E], bf16, kind="ExternalInput").ap()
    projT_d = nc.dram_tensor("projT", [E, E], bf16, kind="ExternalInput").ap()
    ffw2T_d = nc.dram_tensor("ffw2T", [E, E], bf16, kind="ExternalInput").ap()
    bqkv_d = nc.dram_tensor("bqkv", [1, 3 * E], bf16, kind="ExternalInput").ap()
    bproj_d = nc.dram_tensor("bproj", [1, E], bf16, kind="ExternalInput").ap()
    bff2_d = nc.dram_tensor("bff2", [1, E], bf16, kind="ExternalInput").ap()
    g_rep_d = nc.dram_tensor("g_rep", [128, E], f32, kind="ExternalInput").ap()
    b_rep_d = nc.dram_tensor("b_rep", [128, E], f32, kind="ExternalInput").ap()
    out_d = nc.dram_tensor("out", [S, E], f32, kind="ExternalOutput").ap()

    xT_r = xT.rearrange("(t p) s -> p t s", p=128)  # [128, 8, S]
    wqkv_r = wqkvT_d.rearrange("(t p) o -> p t o", p=128)
    proj_r = projT_d.rearrange("(t p) o -> p t o", p=128)
    ffw2_r = ffw2T_d.rearrange("(t p) o -> p t o", p=128)

    with tile.TileContext(nc) as tc:
        import contextlib

        ctx = contextlib.ExitStack()
        with ctx:
            fixed = ctx.enter_context(tc.tile_pool(name="fixed", bufs=1))
            work = ctx.enter_context(tc.tile_pool(name="work", bufs=2))
            work1 = ctx.enter_context(tc.tile_pool(name="work1", bufs=1))
            stats = ctx.enter_context(tc.tile_pool(name="stats", bufs=4))
            psq = ctx.enter_context(tc.tile_pool(name="psq", bufs=3, space="PSUM"))
            psz = ctx.enter_context(tc.tile_pool(name="psz", bufs=1, space="PSUM"))
            psb = ctx.enter_context(tc.tile_pool(name="psb", bufs=2, space="PSUM"))

            # ---- fixed tensors ----
            wqkv_sb = fixed.tile([128, 8, 3 * E], bf16)
            for t in range(8):
                nc.sync.dma_start(out=wqkv_sb[:, t, :], in_=wqkv_r[:, t, :])
            proj_sb = fixed.tile([128, 8, E], bf16)
            ffw2_sb = fixed.tile([128, 8, E], bf16)
            for t in range(8):
                nc.sync.dma_start(out=proj_sb[:, t, :], in_=proj_r[:, t, :])
                nc.sync.dma_start(out=ffw2_sb[:, t, :], in_=ffw2_r[:, t, :])
            bqkv_sb = fixed.tile([1, 3 * E], bf16)
            nc.sync.dma_start(out=bqkv_sb, in_=bqkv_d)
            bproj_sb = fixed.tile([1, E], bf16)
            nc.sync.dma_start(out=bproj_sb, in_=bproj_d)
            bff2_sb = fixed.tile([1, E], bf16)
            nc.sync.dma_start(out=bff2_sb, in_=bff2_d)
            g_rep = fixed.tile([128, E], f32)
            nc.sync.dma_start(out=g_rep, in_=g_rep_d)
            b_rep = fixed.tile([128, E], f32)
            nc.sync.dma_start(out=b_rep, in_=b_rep_d)
            ones_row = fixed.tile([1, 128], bf16)
            nc.vector.memset(ones_row, 1.0)
            ident = fixed.tile([128, 128], bf16)
            make_identity(nc, ident)
            eps_sb = fixed.tile([128, 1], f32)
            nc.vector.memset(eps_sb, EPS)
            invn_sb = fixed.tile([128, 1], f32)
            nc.vector.memset(invn_sb, 1.0 / float(E))
            negone_sb = fixed.tile([128, 1], f32)
            nc.vector.memset(negone_sb, -1.0)

            inv_n = 1.0 / float(E)

            def ln_finish(s1, s2, rs_out, mrs_out):
                """From sum (s1) and sum-of-squares (s2) [128,1] compute
                rsigma and -mu*rsigma."""
                mu = stats.tile([128, 1], f32, tag="mu")
                nc.scalar.mul(mu, s1, inv_n)
                s2n = stats.tile([128, 1], f32, tag="s2n")
                nc.scalar.mul(s2n, s2, inv_n)
                mu2 = stats.tile([128, 1], f32, tag="mu2")
                nc.scalar.square(mu2, mu)
                var = stats.tile([128, 1], f32, tag="var")
                nc.scalar.activation(var, mu2, ACT.Identity, bias=s2n, scale=-1.0)
                lnv = stats.tile([128, 1], f32, tag="lnv")
                nc.scalar.activation(lnv, var, ACT.Ln, bias=eps_sb)
                nc.scalar.activation(rs_out, lnv, ACT.Exp, scale=-0.5)
                nmu = stats.tile([128, 1], f32, tag="nmu")
                nc.scalar.mul(nmu, mu, -1.0)
                nc.gpsimd.tensor_tensor(mrs_out, nmu, rs_out, ALU.mult)

            def qkv_chunks(xf, qkv_sb, chunks):
                for j in chunks:
                    ps = psq.tile([128, 512], f32, tag="psq")
                    for e in range(8):
                        nc.tensor.matmul(
                            ps,
                            xf[:, e, :],
                            wqkv_sb[:, e, j * 512 : (j + 1) * 512],
                            start=(e == 0),
                            stop=False,
                        )
                    nc.tensor.matmul(
                        ps,
                        ones_row,
                        bqkv_sb[:, j * 512 : (j + 1) * 512],
                        start=False,
                        stop=True,
                    )
                    nc.scalar.copy(qkv_sb[:, j * 512 : (j + 1) * 512], ps)

            def qkv_qk(t):
                s0 = t * 128
                xf = work.tile([128, 8, 128], bf16, tag="xf")
                nc.sync.dma_start(out=xf, in_=xT_r[:, :, s0 : s0 + 128])
                qkv_sb = work.tile([128, 3 * E], bf16, tag="qkv")
                qkv_chunks(xf, qkv_sb, (0, 1, 2, 3))
                return xf, qkv_sb

            xf0, qkv_next = qkv_qk(0)
            qkv_chunks(xf0, qkv_next, (4, 5))

            def qk_part(t, qkv_sb):
                q3 = qkv_sb[:, 0:E].rearrange("p (h d) -> p h d", h=H)
                k3 = qkv_sb[:, E : 2 * E].rearrange("p (g d) -> p g d", g=H)
                v3 = qkv_sb[:, 2 * E : 3 * E].rearrange("p (d g) -> p d g", d=DV)

                prod = work1.tile([128, 8192], bf16, tag="prod")
                prod4 = prod.rearrange("p (a g d) -> p a g d", a=8, g=16)
                scr = work1.tile([128, 5376], bf16, tag="scr")
                scores = work.tile([128, H, H], f32, tag="scores")
                for half in range(2):
                    h0 = half * 8
                    qb = q3[:, h0 : h0 + 8, :].unsqueeze(2).broadcast_to([128, 8, 16, 64])
                    kb = k3.unsqueeze(1).broadcast_to([128, 8, 16, 64])
                    nc.vector.tensor_tensor(prod4, kb, qb, ALU.mult)
                    t1 = scr[:, 0:4096].rearrange("p (a g d) -> p a g d", a=8, g=16)
                    nc.vector.tensor_tensor(
                        t1, prod4[:, :, :, 0:32], prod4[:, :, :, 32:64], ALU.add
                    )
                    t2 = prod[:, 0:2048].rearrange("p (a g d) -> p a g d", a=8, g=16)
                    nc.vector.tensor_tensor(
                        t2, t1[:, :, :, 0:16], t1[:, :, :, 16:32], ALU.add
                    )
                    t3 = scr[:, 4096:5120].rearrange("p (a g d) -> p a g d", a=8, g=16)
                    nc.vector.tensor_tensor(
                        t3, t2[:, :, :, 0:8], t2[:, :, :, 8:16], ALU.add
                    )
                    t4 = prod[:, 2048:2560].rearrange("p (a g d) -> p a g d", a=8, g=16)
                    nc.vector.tensor_tensor(
                        t4, t3[:, :, :, 0:4], t3[:, :, :, 4:8], ALU.add
                    )
                    t5 = scr[:, 5120:5376].rearrange("p (a g d) -> p a g d", a=8, g=16)
                    nc.vector.tensor_tensor(
                        t5, t4[:, :, :, 0:2], t4[:, :, :, 2:4], ALU.add
                    )
                    nc.vector.tensor_tensor(
                        scores[:, h0 : h0 + 8, :].unsqueeze(3),
                        t5[:, :, :, 0:1],
                        t5[:, :, :, 1:2],
                        ALU.add,
                    )

                p_sb = work.tile([128, H, H], bf16, tag="p_sb")
                nc.scalar.activation(p_sb, scores, ACT.Exp)
                den = stats.tile([128, H], f32, tag="den")
                nc.vector.tensor_reduce(
                    den, p_sb, axis=mybir.AxisListType.X, op=ALU.add
                )
                rden = stats.tile([128, H], f32, tag="rden")
                nc.vector.reciprocal(rden, den)
                return prod, scr, p_sb, rden, v3

            def av_part(t, prod, scr, p_sb, rden, v3):
                outu = work1.tile([128, H, DV], f32, tag="outu")
                for half in range(2):
                    h0 = half * 8
                    pa = prod.rearrange("p (a d g) -> p a d g", a=8, d=DV)
                    pb = (
                        p_sb[:, h0 : h0 + 8, :]
                        .unsqueeze(2)
                        .broadcast_to([128, 8, 64, 16])
                    )
                    vb = v3.unsqueeze(1).broadcast_to([128, 8, 64, 16])
                    nc.vector.tensor_tensor(pa, vb, pb, ALU.mult)
                    u1 = scr[:, 0:4096].rearrange("p (a d g) -> p a d g", a=8, d=DV)
                    nc.vector.tensor_tensor(
                        u1, pa[:, :, :, 0:8], pa[:, :, :, 8:16], ALU.add
                    )
                    u2 = prod[:, 0:2048].rearrange("p (a d g) -> p a d g", a=8, d=DV)
                    nc.vector.tensor_tensor(
                        u2, u1[:, :, :, 0:4], u1[:, :, :, 4:8], ALU.add
                    )
                    u3 = scr[:, 4096:5120].rearrange("p (a d g) -> p a d g", a=8, d=DV)
                    nc.vector.tensor_tensor(
                        u3, u2[:, :, :, 0:2], u2[:, :, :, 2:4], ALU.add
                    )
                    nc.vector.tensor_tensor(
                        outu[:, h0 : h0 + 8, :].unsqueeze(3),
                        u3[:, :, :, 0:1],
                        u3[:, :, :, 1:2],
                        ALU.add,
                    )

                attn_bf = work.tile([128, E], bf16, tag="attn_bf")
                a3 = attn_bf.rearrange("p (h d) -> p h d", h=H)
                rb = rden.unsqueeze(2).broadcast_to([128, H, DV])
                nc.vector.tensor_tensor(a3, outu, rb, ALU.mult)

                attn_fm = work.tile([128, 8, 128], bf16, tag="attn_fm")
                for e in range(8):
                    nc.sync.dma_start_transpose(
                        attn_fm[:, e, :], attn_bf[:, e * 128 : (e + 1) * 128]
                    )
                return attn_fm

            def tail_pre(t, attn_fm, xp, xbf):
                s0 = t * 128
                # proj + bias + residual accumulated in one 2-bank PSUM tile
                zp = psz.tile([128, 1024], f32, tag="psz")
                for j in range(2):
                    sl = slice(j * 512, (j + 1) * 512)
                    for e in range(8):
                        nc.tensor.matmul(
                            zp[:, sl],
                            attn_fm[:, e, :],
                            proj_sb[:, e, sl],
                            start=(e == 0),
                            stop=False,
                        )
                    nc.tensor.matmul(
                        zp[:, sl], ones_row, bproj_sb[:, sl], start=False, stop=False
                    )
                    nc.tensor.matmul(
                        zp[:, sl], ident, xbf[:, sl], start=False, stop=True
                    )

                lnscr = work1.tile([128, E], bf16, tag="lnscr")
                s1 = stats.tile([128, 1], f32, tag="s1")
                nc.scalar.activation(lnscr, zp, ACT.Identity, accum_out=s1)
                s2 = stats.tile([128, 1], f32, tag="s2")
                nc.scalar.activation(lnscr, zp, ACT.Square, accum_out=s2)
                rs1 = stats.tile([128, 1], f32, tag="rs1")
                mrs1 = stats.tile([128, 1], f32, tag="mrs1")
                ln_finish(s1, s2, rs1, mrs1)
                ln1_bf = work.tile([128, E], bf16, tag="ln1_bf")
                nc.scalar.activation(ln1_bf, zp, ACT.Identity, bias=mrs1, scale=rs1)

                ln1_fm = work.tile([128, 8, 128], bf16, tag="ln1_fm")
                for e in range(8):
                    nc.scalar.dma_start_transpose(
                        ln1_fm[:, e, :], ln1_bf[:, e * 128 : (e + 1) * 128]
                    )
                return ln1_fm

            def tail_post(t, ln1_fm, xp):
                s0 = t * 128
                lnscr = work1.tile([128, E], bf16, tag="lnscr")
                gl = work1.tile([128, E], f32, tag="gl")
                for j in range(2):
                    sl = slice(j * 512, (j + 1) * 512)
                    ps3 = psb.tile([128, 512], f32, tag="psb")
                    for e in range(8):
                        nc.tensor.matmul(
                            ps3,
                            ln1_fm[:, e, :],
                            ffw2_sb[:, e, sl],
                            start=(e == 0),
                            stop=False,
                        )
                    nc.tensor.matmul(
                        ps3, ones_row, bff2_sb[:, sl], start=False, stop=True
                    )
                    nc.scalar.activation(gl[:, sl], ps3, ACT.Gelu)

                z2 = work1.tile([128, E], f32, tag="z2")
                nc.gpsimd.tensor_tensor(z2, gl, xp, ALU.add)

                s1b = stats.tile([128, 1], f32, tag="s1b")
                nc.scalar.activation(lnscr, z2, ACT.Identity, accum_out=s1b)
                s2b = stats.tile([128, 1], f32, tag="s2b")
                nc.scalar.activation(lnscr, z2, ACT.Square, accum_out=s2b)
                rs2 = stats.tile([128, 1], f32, tag="rs2")
                mrs2 = stats.tile([128, 1], f32, tag="mrs2")
                ln_finish(s1b, s2b, rs2, mrs2)
                zn = work1.tile([128, E], f32, tag="gl")
                nc.scalar.activation(zn, z2, ACT.Identity, bias=mrs2, scale=rs2)
                zn2 = work1.tile([128, E], f32, tag="z2")
                nc.gpsimd.tensor_tensor(zn2, zn, g_rep, ALU.mult)
                out_t = work.tile([128, E], f32, tag="out_t")
                nc.gpsimd.tensor_tensor(out_t, zn2, b_rep, ALU.add)
                nc.sync.dma_start(out=out_d[s0 : s0 + 128, :], in_=out_t)

            pending = None  # (t, attn_fm, xp, xbf) awaiting tail
            for t in range(NT):
                s0 = t * 128
                qkv_sb = qkv_next
                qk_state = qk_part(t, qkv_sb)
                nxt = None
                if t + 1 < NT:
                    nxt = qkv_qk(t + 1)
                    qkv_next = nxt[1]
                    qkv_chunks(nxt[0], nxt[1], (4, 5))
                xp = work.tile([128, E], f32, tag="xp")
                nc.sync.dma_start(out=xp, in_=x_pm[s0 : s0 + 128, :])
                xbf = work.tile([128, E], bf16, tag="xbf")
                nc.sync.dma_start(out=xbf, in_=x_bf[s0 : s0 + 128, :])

                if pending is not None:
                    ln1_fm_p = tail_pre(*pending)
                attn_fm = av_part(t, *qk_state)
                if pending is not None:
                    tail_post(pending[0], ln1_fm_p, pending[2])
                pending = (t, attn_fm, xp, xbf)
            ln1_fm_p = tail_pre(*pending)
            tail_post(pending[0], ln1_fm_p, pending[2])

    _split_excess_waits(nc)
    return nc


def _host_prep(inputs):
    x = np.asarray(inputs["x"], np.float32)
    qk_w = np.asarray(inputs["qk_w"], np.float32)
    qk_b = np.asarray(inputs["qk_b"], np.float32)
    v_w = np.asarray(inputs["v_w"], np.float32)
    v_b = np.asarray(inputs["v_b"], np.float32)
    proj_w = np.asarray(inputs["proj_w"], np.float32)
    proj_b = np.asarray(inputs["proj_b"], np.float32)
    ff_w = np.asarray(inputs["ff_w"], np.float32)
    ff_b = np.asarray(inputs["ff_b"], np.float32)
    ln_g = np.asarray(inputs["ln_g"], np.float32)
    ln_b = np.asarray(inputs["ln_b"], np.float32)

    scale = np.float32(1.0 / np.sqrt(DQ))
    Wq = qk_w[:E] * scale
    bq = qk_b[:E] * scale
    Wk = qk_w[E:]
    bk = qk_b[E:]
    g_idx, d_idx = np.meshgrid(np.arange(H), np.arange(DV), indexing="ij")
    perm = np.empty(E, np.int64)
    perm[(d_idx * H + g_idx).ravel()] = (g_idx * DV + d_idx).ravel()
    Wv2 = v_w[perm]
    bv2 = v_b[perm]

    wqkvT = np.ascontiguousarray(np.concatenate([Wq, Wk, Wv2], 0).T.astype(BF))
    bqkv = np.concatenate([bq, bk, bv2])[None, :].astype(BF)
    projT = np.ascontiguousarray(proj_w.T.astype(BF))
    bproj = proj_b[None, :].astype(BF)
    ffw2T = np.ascontiguousarray((ff_w * ln_g[None, :]).T.astype(BF))
    bff2 = (ff_b + ff_w @ ln_b)[None, :].astype(BF)
    g_rep = np.ascontiguousarray(np.broadcast_to(ln_g[None, :], (128, E)), np.float32)
    b_rep = np.ascontiguousarray(np.broadcast_to(ln_b[None, :], (128, E)), np.float32)

    shared = {
        "wqkvT": wqkvT,
        "bqkv": bqkv,
        "projT": projT,
        "bproj": bproj,
        "ffw2T": ffw2T,
        "bff2": bff2,
        "g_rep": g_rep,
        "b_rep": b_rep,
    }
    in_maps = []
    for b in range(B):
        xb = np.ascontiguousarray(x[b])  # [S, E] f32
        m = {
            "x_pm": xb,
            "x_bf": xb.astype(BF),
            "xT": np.ascontiguousarray(xb.T.astype(BF)),
        }
        m.update(shared)
        in_maps.append(m)
    return in_maps


def kernel(**inputs) -> np.ndarray:
    from concourse.bass_utils import run_bass_kernel_spmd

    if "nc" not in _CACHE:
        _CACHE["nc"] = _build_program()
    nc = _CACHE["nc"]

    in_maps = _host_prep(inputs)
    res = run_bass_kernel_spmd(nc, in_maps, core_ids=list(range(B)))
    out = np.stack([res.results[b]["out"] for b in range(B)], 0)
    return out.astype(np.float32)
